# revision 12
# baseline (speedup 1.0000x reference)
"""Multi-head causal attention (B=4, T=2048, C=1024, H=16) on 8 trn2 cores.

Sharding: core c handles batch b=c//2 and head-group hg=c%2 (8 heads).
Each core computes qkv projection for its heads, causal attention, and a
partial output projection; the host sums the two head-group partials per
batch. No collectives.

On-chip dataflow (per core, all fp16 operands / fp32 PSUM):
  x [T,C] --xbar-dma-transpose--> xT [C,T]
  qT/kT = (W.T x.T) feature-major [512, T]   (head-pair tiles [128, T])
  v     = (x W) natural [T, 512] interleaved with a ones column per head
  scores^T [k,q] per head pair via row-tiled K=64 matmul packing
  expS = ACT exp(0.125*s) -> fp16, causal mask on diagonal tiles only
  AV: psum[65, q] += v_aug[k,65].T @ expS[k,q]  (row 64 = softmax denom)
  o^T = numerator * bcast(1/denom)  folded into PSUM evacuation
  out[t, :] += o^T.T @ w_proj  (natural layout, fp32 DMA out)
"""

import numpy as np
import ml_dtypes

B, T, C, H = 4, 2048, 1024, 16
HD = C // H          # 64
HG = H // 2          # 8 heads per core
CPG = HG * HD        # 512 feature cols per head-group
NCORES = 8
NCT = C // 128       # 8 contraction tiles over C
NTT = T // 128       # 16 token tiles
NCH = T // 512       # 4 query chunks
VROW = HG * (HD + 1)  # 520: v row layout, 65 cols per head (64 v + 1 ones)

_cache = {}


def _build_nc(dump=False):
    import concourse.mybir as mybir
    import concourse.tile as tile
    from concourse import bacc

    f16 = mybir.dt.float16
    f32 = mybir.dt.float32
    mult = mybir.AluOpType.mult
    Exp = mybir.ActivationFunctionType.Exp

    nc = bacc.Bacc(None, target_bir_lowering=False, debug=False)

    x_d = nc.dram_tensor("x", [T, C], f16, kind="ExternalInput")
    wq_d = nc.dram_tensor("wq", [C, 3 * CPG], f16, kind="ExternalInput")
    wp_d = nc.dram_tensor("wp", [CPG, C], f16, kind="ExternalInput")
    mask_d = nc.dram_tensor("mask", [128, 2048], f16, kind="ExternalInput")
    out_d = nc.dram_tensor("out", [T, C], f32, kind="ExternalOutput")
    dump_d = {}
    if dump:
        for name, cols in [("xT", NCT * T), ("qT", 4 * T), ("kT", 4 * T),
                           ("v_sb", NTT * VROW), ("oT", 4 * T)]:
            dump_d[name] = nc.dram_tensor(f"dump_{name}", [128, cols], f16,
                                          kind="ExternalOutput")
        for name, cols in [("s", 2 * 512), ("av", 2 * 512), ("rr", 2 * 1024)]:
            dump_d[name] = nc.dram_tensor(f"dump_{name}", [128, cols], f32,
                                          kind="ExternalOutput")
        dump_d["e"] = nc.dram_tensor("dump_e", [128, 8 * 512], f16,
                                     kind="ExternalOutput")

    with tile.TileContext(nc) as tc:
        with (
            tc.tile_pool(name="persist", bufs=1) as pp,
            tc.tile_pool(name="work", bufs=3) as wp_pool,
            tc.tile_pool(name="psmm", bufs=2, space="PSUM") as ps_mm,
            tc.tile_pool(name="pssc", bufs=2, space="PSUM") as ps_sc,
            tc.tile_pool(name="psav", bufs=1, space="PSUM") as ps_av,
        ):
            # ---- persistent SBUF tensors ----
            xT = pp.tile([128, NCT * T], f16)          # [c-tile, t] 32KB/part
            w_sb = pp.tile([128, NCT * 3 * CPG], f16)  # qkv weights 24KB/part
            wp_sb = pp.tile([128, 4 * C], f16)         # proj weights 8KB/part
            mask_sb = pp.tile([128, 2048], f16)
            qT = pp.tile([128, 4 * T], f16)            # 4 head-pair tiles
            kT = pp.tile([128, 4 * T], f16)
            v_sb = pp.tile([128, NTT * VROW], f16)     # v + ones cols
            oT = pp.tile([128, 4 * T], f16)            # attn out, pair tiles

            # ---- input DMA ----
            for ct in range(NCT):
                nc.sync.dma_start(
                    out=w_sb[:, ct * 3 * CPG:(ct + 1) * 3 * CPG],
                    in_=wq_d[ct * 128:(ct + 1) * 128, :],
                )
                nc.sync.dma_start_transpose(
                    out=xT[:, ct * T:(ct + 1) * T],
                    in_=x_d[:, ct * 128:(ct + 1) * 128],
                )
            for cp in range(4):
                nc.sync.dma_start(
                    out=wp_sb[:, cp * C:(cp + 1) * C],
                    in_=wp_d[cp * 128:(cp + 1) * 128, :],
                )
            nc.sync.dma_start(out=mask_sb[:], in_=mask_d[:])

            # ones columns for the softmax-denominator trick
            ones_view = v_sb.rearrange("p (n d) -> p n d", d=HD + 1)[:, :, HD:HD + 1]
            nc.vector.memset(ones_view, 1.0)

            # ---- stage 1: qT / kT  (feature-major) ----
            # qkvT[c', t] tiles: c'-tile cpt covers q (cpt 0-3) / k (cpt 4-7)
            for cpt in range(8):
                dst = qT if cpt < 4 else kT
                pair = cpt % 4
                for tch in range(NCH):
                    ps = ps_mm.tile([128, 512], f32, tag="mm")
                    for ct in range(NCT):
                        nc.tensor.matmul(
                            ps[:],
                            lhsT=w_sb[:, ct * 3 * CPG + cpt * 128:ct * 3 * CPG + cpt * 128 + 128],
                            rhs=xT[:, ct * T + tch * 512:ct * T + tch * 512 + 512],
                            start=(ct == 0), stop=(ct == NCT - 1),
                        )
                    nc.vector.tensor_scalar_mul(
                        dst[:, pair * T + tch * 512:pair * T + tch * 512 + 512], ps[:], 1.0
                    )

            # ---- stage 1b: v natural [t, 512] with ones interleave ----
            for tt in range(NTT):
                ps = ps_mm.tile([128, 512], f32, tag="mm")
                for ct in range(NCT):
                    nc.tensor.matmul(
                        ps[:],
                        lhsT=xT[:, ct * T + tt * 128:ct * T + tt * 128 + 128],
                        rhs=w_sb[:, ct * 3 * CPG + 2 * CPG:(ct + 1) * 3 * CPG],
                        start=(ct == 0), stop=(ct == NCT - 1),
                    )
                vdst = v_sb.rearrange("p (n d) -> p n d", d=HD + 1)[
                    :, tt * HG:(tt + 1) * HG, 0:HD
                ]
                nc.vector.tensor_scalar_mul(
                    vdst, ps[:].rearrange("p (h d) -> p h d", d=HD), 1.0
                )

            # ---- stage 2: attention per head pair ----
            for p in range(4):
                hA, hB = 2 * p, 2 * p + 1
                for ci in range(NCH):
                    jmax = 4 * ci + 3
                    av_a = ps_av.tile([65, 512], f32, tag="avA")
                    av_b = ps_av.tile([65, 512], f32, tag="avB")
                    for j in range(jmax + 1):
                        s_a = ps_sc.tile([128, 512], f32, tag="sA")
                        s_b = ps_sc.tile([128, 512], f32, tag="sB")
                        nc.tensor.matmul(
                            s_a[:],
                            lhsT=kT[0:64, p * T + j * 128:p * T + j * 128 + 128],
                            rhs=qT[0:64, p * T + ci * 512:p * T + ci * 512 + 512],
                            start=True, stop=True,
                        )
                        nc.tensor.matmul(
                            s_b[:],
                            lhsT=kT[64:128, p * T + j * 128:p * T + j * 128 + 128],
                            rhs=qT[64:128, p * T + ci * 512:p * T + ci * 512 + 512],
                            start=True, stop=True,
                        )
                        if dump and p == 0 and ci == 0 and j == 0:
                            sdmp = wp_pool.tile([128, 1024], f32, tag="sdmp")
                            nc.vector.tensor_scalar_mul(sdmp[:, 0:512], s_a[:], 1.0)
                            nc.vector.tensor_scalar_mul(sdmp[:, 512:1024], s_b[:], 1.0)
                            nc.sync.dma_start(out=dump_d["s"][:], in_=sdmp[:])
                        e_a = wp_pool.tile([128, 512], f16, tag="eA")
                        e_b = wp_pool.tile([128, 512], f16, tag="eB")
                        nc.scalar.activation(e_a[:], s_a[:], Exp, scale=0.125)
                        nc.scalar.activation(e_b[:], s_b[:], Exp, scale=0.125)
                        r = j - 4 * ci
                        if r >= 0:  # diagonal tile: causal mask
                            nc.vector.scalar_tensor_tensor(
                                e_a[:], e_a[:], 1.0,
                                mask_sb[:, r * 512:r * 512 + 512], op0=mult, op1=mult,
                            )
                            nc.vector.scalar_tensor_tensor(
                                e_b[:], e_b[:], 1.0,
                                mask_sb[:, r * 512:r * 512 + 512], op0=mult, op1=mult,
                            )
                        nc.tensor.matmul(
                            av_a[:],
                            lhsT=v_sb[:, j * VROW + hA * 65:j * VROW + hA * 65 + 65],
                            rhs=e_a[:],
                            start=(j == 0), stop=(j == jmax),
                        )
                        nc.tensor.matmul(
                            av_b[:],
                            lhsT=v_sb[:, j * VROW + hB * 65:j * VROW + hB * 65 + 65],
                            rhs=e_b[:],
                            start=(j == 0), stop=(j == jmax),
                        )
                        if dump and p == 0 and ci == 0:
                            nc.sync.dma_start(
                                out=dump_d["e"][:, j * 512:(j + 1) * 512], in_=e_a[:])
                            nc.sync.dma_start(
                                out=dump_d["e"][:, (4 + j) * 512:(4 + j + 1) * 512], in_=e_b[:])
                    if dump and p == 0 and ci == 0:
                        avdmp = wp_pool.tile([65, 1024], f32, tag="avdmp")
                        nc.vector.tensor_scalar_mul(avdmp[:, 0:512], av_a[:], 1.0)
                        nc.vector.tensor_scalar_mul(avdmp[:, 512:1024], av_b[:], 1.0)
                        nc.sync.dma_start(out=dump_d["av"][0:65, :], in_=avdmp[:])
                    # evacuate: divide by denominator (psum row 64)
                    recip = wp_pool.tile([65, 1024], f32, tag="recip")
                    nc.vector.reciprocal(recip[64:65, 0:512], av_a[64:65, :])
                    nc.vector.reciprocal(recip[64:65, 512:1024], av_b[64:65, :])
                    # partition_broadcast reads the wrong partition for
                    # base>0 on HW: bounce the recip row to partition 0 first
                    recip0 = wp_pool.tile([1, 1024], f32, tag="recip0")
                    nc.sync.dma_start(out=recip0[:], in_=recip[64:65, :])
                    rbc = wp_pool.tile([64, 1024], f32, tag="rbc")
                    nc.gpsimd.partition_broadcast(rbc[0:64, 0:512], recip0[0:1, 0:512])
                    nc.gpsimd.partition_broadcast(rbc[0:64, 512:1024], recip0[0:1, 512:1024])
                    if dump and p == 0 and ci == 0:
                        nc.sync.dma_start(out=dump_d["rr"][0:65, 0:1024], in_=recip[:])
                        nc.sync.dma_start(out=dump_d["rr"][0:64, 1024:2048], in_=rbc[:])
                    nc.vector.scalar_tensor_tensor(
                        oT[0:64, p * T + ci * 512:p * T + ci * 512 + 512],
                        av_a[0:64, :], 1.0, rbc[0:64, 0:512], op0=mult, op1=mult,
                    )
                    tmpb = wp_pool.tile([64, 512], f16, tag="tmpb")
                    nc.vector.scalar_tensor_tensor(
                        tmpb[:], av_b[0:64, :], 1.0, rbc[0:64, 512:1024],
                        op0=mult, op1=mult,
                    )
                    # shift head-B rows to partitions 64-127 of the pair tile
                    nc.sync.dma_start(
                        out=oT[64:128, p * T + ci * 512:p * T + ci * 512 + 512],
                        in_=tmpb[:],
                    )

            # ---- stage 3: output projection (natural [t, out]) ----
            for tt in range(NTT):
                for oc in range(2):
                    ps = ps_mm.tile([128, 512], f32, tag="mm")
                    for cp in range(4):
                        nc.tensor.matmul(
                            ps[:],
                            lhsT=oT[:, cp * T + tt * 128:cp * T + tt * 128 + 128],
                            rhs=wp_sb[:, cp * C + oc * 512:cp * C + oc * 512 + 512],
                            start=(cp == 0), stop=(cp == 3),
                        )
                    ot = wp_pool.tile([128, 512], f32, tag="ostage")
                    nc.vector.tensor_scalar_mul(ot[:], ps[:], 1.0)
                    nc.sync.dma_start(
                        out=out_d[tt * 128:(tt + 1) * 128, oc * 512:(oc + 1) * 512],
                        in_=ot[:],
                    )

            if dump:
                for name, sb in [("xT", xT), ("qT", qT), ("kT", kT),
                                 ("v_sb", v_sb), ("oT", oT)]:
                    nc.sync.dma_start(out=dump_d[name][:], in_=sb[:])

    nc.compile()
    return nc


def get_nc():
    if "nc" not in _cache:
        _cache["nc"] = _build_nc()
    return _cache["nc"]


def make_mask():
    # mask[r][k, q] = 1 if 128*r + k <= q else 0, r = 0..3 side by side
    k = np.arange(128)[:, None]
    q = np.arange(512)[None, :]
    cols = [(128 * r + k <= q) for r in range(4)]
    return np.concatenate(cols, axis=1).astype(np.float16)


def make_in_maps(x, w_qkv, w_proj):
    f16 = np.float16
    mask = make_mask()
    in_maps = []
    for c in range(NCORES):
        b, hg = c // 2, c % 2
        cols = np.concatenate([
            np.arange(hg * CPG, hg * CPG + CPG),
            np.arange(C + hg * CPG, C + hg * CPG + CPG),
            np.arange(2 * C + hg * CPG, 2 * C + hg * CPG + CPG),
        ])
        in_maps.append({
            "x": np.ascontiguousarray(x[b]).astype(f16),
            "wq": np.ascontiguousarray(w_qkv[:, cols]).astype(f16),
            "wp": np.ascontiguousarray(w_proj[hg * CPG:(hg + 1) * CPG, :]).astype(f16),
            "mask": mask,
        })
    return in_maps


def kernel(x, w_qkv, w_proj, **run_kwargs):
    from concourse.bass_utils import run_bass_kernel_spmd

    x = np.asarray(x)
    nc = get_nc()
    in_maps = make_in_maps(x, np.asarray(w_qkv), np.asarray(w_proj))
    res = run_bass_kernel_spmd(nc, in_maps, list(range(NCORES)), **run_kwargs)
    _cache["last_results"] = res
    out = np.empty((B, T, C), np.float32)
    for b in range(B):
        out[b] = res.results[2 * b]["out"] + res.results[2 * b + 1]["out"]
    return out


# revision 14
# speedup vs baseline: 1.4152x; 1.4152x over previous
"""Multi-head causal attention (B=4, T=2048, C=1024, H=16) on 8 trn2 cores.

Sharding: core c handles batch b=c//2 and head-group hg=c%2 (8 heads).
Each core computes qkv projection for its heads, causal attention, and a
partial output projection; the host sums the two head-group partials per
batch. No collectives.

On-chip dataflow (per core, all fp16 operands / fp32 PSUM):
  x [T,C] --xbar-dma-transpose--> xT [C,T]
  qT/kT = (W.T x.T) feature-major [512, T]   (head-pair tiles [128, T])
  v     = (x W) natural [T, 512] interleaved with a ones column per head
  scores^T [k,q] per head pair via row-tiled K=64 matmul packing, the two
    heads' tiles packed side by side in one [128, 1024] 2-bank PSUM tile
  expS = one ACT exp(0.125*s) per [128,1024] tile -> fp16, causal mask on
    diagonal tiles only (mask input pre-doubled to 1024 wide)
  AV: psum[65, q] += v_aug[k,65].T @ expS[k,q]  (row 64 = softmax denom)
  o^T = numerator * bcast(1/denom)  folded into PSUM evacuation; the
    reciprocal runs on a DMA-packed [64,16] layout (64 lanes, not 1)
  out[t, :] += o^T.T @ w_proj  (natural layout, fp32 DMA out)
"""

import numpy as np

B, T, C, H = 4, 2048, 1024, 16
HD = C // H          # 64
HG = H // 2          # 8 heads per core
CPG = HG * HD        # 512 feature cols per head-group
NCORES = 8
NCT = C // 128       # 8 contraction tiles over C
NTT = T // 128       # 16 token tiles
NCH = T // 512       # 4 query chunks
VROW = HG * (HD + 1)  # 520: v row layout, 65 cols per head (64 v + 1 ones)

_cache = {}


def _build_nc(dump=False):
    import concourse.mybir as mybir
    import concourse.tile as tile
    from concourse import bacc

    f16 = mybir.dt.float16
    f32 = mybir.dt.float32
    mult = mybir.AluOpType.mult
    Exp = mybir.ActivationFunctionType.Exp

    nc = bacc.Bacc(None, target_bir_lowering=False, debug=False)

    x_d = nc.dram_tensor("x", [T, C], f16, kind="ExternalInput")
    wq_d = nc.dram_tensor("wq", [C, 3 * CPG], f16, kind="ExternalInput")
    wp_d = nc.dram_tensor("wp", [CPG, C], f16, kind="ExternalInput")
    mask_d = nc.dram_tensor("mask", [128, 4096], f16, kind="ExternalInput")
    out_d = nc.dram_tensor("out", [T, C], f32, kind="ExternalOutput")
    dump_d = {}
    if dump:
        for name, cols in [("xT", NCT * T), ("qT", 4 * T), ("kT", 4 * T),
                           ("v_sb", NTT * VROW), ("oT", 4 * T)]:
            dump_d[name] = nc.dram_tensor(f"dump_{name}", [128, cols], f16,
                                          kind="ExternalOutput")

    with tile.TileContext(nc) as tc:
        with (
            tc.tile_pool(name="persist", bufs=1) as pp,
            tc.tile_pool(name="work", bufs=4) as wk,
            tc.tile_pool(name="psum", bufs=1, space="PSUM") as psp,
        ):
            # ---- persistent SBUF tensors ----
            xT = pp.tile([128, NCT * T], f16)          # [c-tile, t] 32KB/part
            w_sb = pp.tile([128, NCT * 3 * CPG], f16)  # qkv weights 24KB/part
            wp_sb = pp.tile([128, 4 * C], f16)         # proj weights 8KB/part
            mask_sb = pp.tile([128, 4096], f16)
            qT = pp.tile([128, 4 * T], f16)            # 4 head-pair tiles
            kT = pp.tile([128, 4 * T], f16)
            v_sb = pp.tile([128, NTT * VROW], f16)     # v + ones cols
            oT = pp.tile([128, 4 * T], f16)            # attn out, pair tiles

            # ---- input DMA ----
            for ct in range(NCT):
                nc.sync.dma_start(
                    out=w_sb[:, ct * 3 * CPG:(ct + 1) * 3 * CPG],
                    in_=wq_d[ct * 128:(ct + 1) * 128, :],
                )
                nc.sync.dma_start_transpose(
                    out=xT[:, ct * T:(ct + 1) * T],
                    in_=x_d[:, ct * 128:(ct + 1) * 128],
                )
            for cp in range(4):
                nc.sync.dma_start(
                    out=wp_sb[:, cp * C:(cp + 1) * C],
                    in_=wp_d[cp * 128:(cp + 1) * 128, :],
                )
            nc.sync.dma_start(out=mask_sb[:], in_=mask_d[:])

            # ones columns for the softmax-denominator trick
            ones_view = v_sb.rearrange("p (n d) -> p n d", d=HD + 1)[:, :, HD:HD + 1]
            nc.vector.memset(ones_view, 1.0)

            # ---- stage 1: qT / kT  (feature-major) ----
            for cpt in range(8):
                dst = qT if cpt < 4 else kT
                pair = cpt % 4
                for tch in range(NCH):
                    ps = psp.tile([128, 512], f32, tag="s", bufs=2)
                    for ct in range(NCT):
                        nc.tensor.matmul(
                            ps[:],
                            lhsT=w_sb[:, ct * 3 * CPG + cpt * 128:ct * 3 * CPG + cpt * 128 + 128],
                            rhs=xT[:, ct * T + tch * 512:ct * T + tch * 512 + 512],
                            start=(ct == 0), stop=(ct == NCT - 1),
                        )
                    nc.vector.tensor_scalar_mul(
                        dst[:, pair * T + tch * 512:pair * T + tch * 512 + 512], ps[:], 1.0
                    )

            # ---- stage 1b: v natural [t, 512] with ones interleave ----
            for tt in range(NTT):
                ps = psp.tile([128, 512], f32, tag="s", bufs=2)
                for ct in range(NCT):
                    nc.tensor.matmul(
                        ps[:],
                        lhsT=xT[:, ct * T + tt * 128:ct * T + tt * 128 + 128],
                        rhs=w_sb[:, ct * 3 * CPG + 2 * CPG:(ct + 1) * 3 * CPG],
                        start=(ct == 0), stop=(ct == NCT - 1),
                    )
                vdst = v_sb.rearrange("p (n d) -> p n d", d=HD + 1)[
                    :, tt * HG:(tt + 1) * HG, 0:HD
                ]
                nc.vector.tensor_scalar_mul(
                    vdst, ps[:].rearrange("p (h d) -> p h d", d=HD), 1.0
                )

            # ---- stage 2: attention per head pair ----
            for p in range(4):
                hA, hB = 2 * p, 2 * p + 1
                for ci in range(NCH):
                    jmax = 4 * ci + 3
                    av_a = psp.tile([65, 512], f32, tag="av", bufs=4)
                    av_b = psp.tile([65, 512], f32, tag="av", bufs=4)
                    for j in range(jmax + 1):
                        s_ab = psp.tile([128, 1024], f32, tag="s", bufs=2)
                        nc.tensor.matmul(
                            s_ab[:, 0:512],
                            lhsT=kT[0:64, p * T + j * 128:p * T + j * 128 + 128],
                            rhs=qT[0:64, p * T + ci * 512:p * T + ci * 512 + 512],
                            start=True, stop=True,
                        )
                        nc.tensor.matmul(
                            s_ab[:, 512:1024],
                            lhsT=kT[64:128, p * T + j * 128:p * T + j * 128 + 128],
                            rhs=qT[64:128, p * T + ci * 512:p * T + ci * 512 + 512],
                            start=True, stop=True,
                        )
                        e_ab = wk.tile([128, 1024], f16, tag="e")
                        nc.scalar.activation(e_ab[:], s_ab[:], Exp, scale=0.125)
                        r = j - 4 * ci
                        if r >= 0:  # diagonal tile: causal mask (pre-doubled)
                            nc.vector.scalar_tensor_tensor(
                                e_ab[:], e_ab[:], 1.0,
                                mask_sb[:, r * 1024:r * 1024 + 1024], op0=mult, op1=mult,
                            )
                        nc.tensor.matmul(
                            av_a[:],
                            lhsT=v_sb[:, j * VROW + hA * 65:j * VROW + hA * 65 + 65],
                            rhs=e_ab[:, 0:512],
                            start=(j == 0), stop=(j == jmax),
                        )
                        nc.tensor.matmul(
                            av_b[:],
                            lhsT=v_sb[:, j * VROW + hB * 65:j * VROW + hB * 65 + 65],
                            rhs=e_ab[:, 512:1024],
                            start=(j == 0), stop=(j == jmax),
                        )
                    # evacuate: divide by denominator (psum row 64).
                    # Single-lane reciprocal is ~3.3us; pack the 1024 denoms
                    # into 64 lanes via DMA round-trip instead.
                    den = wk.tile([1, 1024], f32, tag="den")
                    nc.vector.tensor_scalar_mul(den[0:1, 0:512], av_a[64:65, :], 1.0)
                    nc.vector.tensor_scalar_mul(den[0:1, 512:1024], av_b[64:65, :], 1.0)
                    denp = wk.tile([64, 16], f32, tag="denp")
                    nc.sync.dma_start(out=denp[:], in_=den[:])
                    recp = wk.tile([64, 16], f32, tag="recp")
                    nc.vector.reciprocal(recp[:], denp[:])
                    recip0 = wk.tile([1, 1024], f32, tag="recip0")
                    nc.sync.dma_start(out=recip0[:], in_=recp[:])
                    rbc = wk.tile([64, 1024], f32, tag="rbc")
                    nc.gpsimd.partition_broadcast(rbc[0:64, 0:512], recip0[0:1, 0:512])
                    nc.gpsimd.partition_broadcast(rbc[0:64, 512:1024], recip0[0:1, 512:1024])
                    nc.vector.scalar_tensor_tensor(
                        oT[0:64, p * T + ci * 512:p * T + ci * 512 + 512],
                        av_a[0:64, :], 1.0, rbc[0:64, 0:512], op0=mult, op1=mult,
                    )
                    tmpb = wk.tile([64, 512], f16, tag="tmpb")
                    nc.vector.scalar_tensor_tensor(
                        tmpb[:], av_b[0:64, :], 1.0, rbc[0:64, 512:1024],
                        op0=mult, op1=mult,
                    )
                    # shift head-B rows to partitions 64-127 of the pair tile
                    nc.sync.dma_start(
                        out=oT[64:128, p * T + ci * 512:p * T + ci * 512 + 512],
                        in_=tmpb[:],
                    )

            # ---- stage 3: output projection (natural [t, out]) ----
            for tt in range(NTT):
                for oc in range(2):
                    ps = psp.tile([128, 512], f32, tag="s", bufs=2)
                    for cp in range(4):
                        nc.tensor.matmul(
                            ps[:],
                            lhsT=oT[:, cp * T + tt * 128:cp * T + tt * 128 + 128],
                            rhs=wp_sb[:, cp * C + oc * 512:cp * C + oc * 512 + 512],
                            start=(cp == 0), stop=(cp == 3),
                        )
                    ot = wk.tile([128, 512], f32, tag="ostage")
                    nc.vector.tensor_scalar_mul(ot[:], ps[:], 1.0)
                    nc.sync.dma_start(
                        out=out_d[tt * 128:(tt + 1) * 128, oc * 512:(oc + 1) * 512],
                        in_=ot[:],
                    )

            if dump:
                for name, sb in [("xT", xT), ("qT", qT), ("kT", kT),
                                 ("v_sb", v_sb), ("oT", oT)]:
                    nc.sync.dma_start(out=dump_d[name][:], in_=sb[:])

    nc.compile()
    return nc


def get_nc():
    if "nc" not in _cache:
        _cache["nc"] = _build_nc()
    return _cache["nc"]


def make_mask():
    # mask[r][k, q] = 1 if 128*r + k <= q else 0; each r-block doubled to
    # 1024 wide so one multiply covers both heads of a packed pair.
    k = np.arange(128)[:, None]
    q = np.arange(512)[None, :]
    cols = []
    for r in range(4):
        m = (128 * r + k <= q)
        cols += [m, m]
    return np.concatenate(cols, axis=1).astype(np.float16)


def make_in_maps(x, w_qkv, w_proj):
    f16 = np.float16
    mask = make_mask()
    in_maps = []
    for c in range(NCORES):
        b, hg = c // 2, c % 2
        cols = np.concatenate([
            np.arange(hg * CPG, hg * CPG + CPG),
            np.arange(C + hg * CPG, C + hg * CPG + CPG),
            np.arange(2 * C + hg * CPG, 2 * C + hg * CPG + CPG),
        ])
        in_maps.append({
            "x": np.ascontiguousarray(x[b]).astype(f16),
            "wq": np.ascontiguousarray(w_qkv[:, cols]).astype(f16),
            "wp": np.ascontiguousarray(w_proj[hg * CPG:(hg + 1) * CPG, :]).astype(f16),
            "mask": mask,
        })
    return in_maps


def kernel(x, w_qkv, w_proj, **run_kwargs):
    from concourse.bass_utils import run_bass_kernel_spmd

    x = np.asarray(x)
    nc = get_nc()
    in_maps = make_in_maps(x, np.asarray(w_qkv), np.asarray(w_proj))
    res = run_bass_kernel_spmd(nc, in_maps, list(range(NCORES)), **run_kwargs)
    _cache["last_results"] = res
    out = np.empty((B, T, C), np.float32)
    for b in range(B):
        out[b] = res.results[2 * b]["out"] + res.results[2 * b + 1]["out"]
    return out
